# revision 1
# baseline (speedup 1.0000x reference)
"""Trainium2 Bass kernel for nn_MultiHeadAttention_63986422775834.

Computation (see harness reference):
    q = x @ Wq + bq; k = x @ Wk + bk; v = x @ Wv + bv          # [N, D]
    group rows by 8: scores[b,h,g] = q[8b+h] . k[8b+g] / sqrt(D)
    w = softmax(scores, axis=-1);  out[8b+h] = sum_g w[b,h,g] * v[8b+g]

Sharding: data-parallel over rows across 8 NeuronCores (2048 rows each;
row groups of 8 never cross a shard boundary). Weights replicated.

Per-core kernel (bf16 matmuls, fp32 accumulate):
  phase A: load x strips, cast bf16, PE-transpose -> resident xT tiles
           (d_in on partitions).
  pass 1:  stream Wq/Wk in d_out chunks; qT/kT = W.T-oriented projection
           GEMMs (d_out on partitions); S[128x128 row-block diag tiles]
           accumulated in SBUF over d_out chunks; masked softmax over
           8x8 diagonal blocks; PE-transpose the softmax weights.
  pass 2:  stream Wv; V tiles (rows on partitions); O = w @ V + bv; DMA out.

DMA emission order doubles as ring priority: first W chunks are hoisted,
chunk loads use one-chunk lookahead, and W is read in k-grouped slabs so
segments are 1-2KB. The startup (~8MB of prerequisites) is aggregate-DMA-
bandwidth-bound at ~20us of PE idle; measured plateau ~795us, MFU ~83%.
"""

import sys

sys.path.insert(0, "/opt/trn_rl_repo")

import numpy as np
import ml_dtypes

import concourse.mybir as mybir
import concourse.tile as tile
from concourse import bacc
from concourse.bass_utils import run_bass_kernel_spmd

# problem shape (hardcoded per contract)
N_FULL = 16384
D = 2048
H = 8
N_CORES = 8
R = N_FULL // N_CORES  # rows per core = 2048
P = 128
KO = D // P  # 16 k-subtiles along d_in
SCALE = 1.0 / np.sqrt(np.float32(D))

BF16 = mybir.dt.bfloat16
F32 = mybir.dt.float32

# row blocks (row0, nrows): small first block -> compute starts early;
# small last block -> output drain starts early
BLOCKS = [(0, 512), (512, 512), (1024, 512), (1536, 512)]
# pass-1 d_out chunks (col0, width): small first chunks for startup
CHUNKS1 = [(256 * i, 256) for i in range(8)]
# pass-2 d_out chunks
CHUNKS2 = [(0, 512), (512, 512), (1024, 512), (1536, 256), (1792, 256)]

assert sum(n for _, n in BLOCKS) == R
assert sum(w for _, w in CHUNKS1) == D
assert sum(w for _, w in CHUNKS2) == D


def build_program():
    nc = bacc.Bacc("TRN2", target_bir_lowering=False, debug=False, num_devices=N_CORES)

    xs = nc.dram_tensor("xs", [R, D], F32, kind="ExternalInput")
    Wq = nc.dram_tensor("Wq", [D, D], F32, kind="ExternalInput")
    Wk = nc.dram_tensor("Wk", [D, D], F32, kind="ExternalInput")
    Wv = nc.dram_tensor("Wv", [D, D], F32, kind="ExternalInput")
    bqt = nc.dram_tensor("bqt", [P, KO], F32, kind="ExternalInput")
    bkt = nc.dram_tensor("bkt", [P, KO], F32, kind="ExternalInput")
    bvr = nc.dram_tensor("bvr", [P, D], F32, kind="ExternalInput")
    maskt = nc.dram_tensor("maskt", [P, P], F32, kind="ExternalInput")
    ident = nc.dram_tensor("ident", [P, P], BF16, kind="ExternalInput")
    out = nc.dram_tensor("out", [R, D], F32, kind="ExternalOutput")

    # d_in-major views of the weights: w[p, ko, n] = W[ko*128+p, n]
    wq_ap = Wq[:].rearrange("(ko p) n -> p ko n", p=P)
    wk_ap = Wk[:].rearrange("(ko p) n -> p ko n", p=P)
    wv_ap = Wv[:].rearrange("(ko p) n -> p ko n", p=P)

    with tile.TileContext(nc) as tc:
        with (
            tc.tile_pool(name="const", bufs=1) as const,
            tc.tile_pool(name="xT", bufs=1) as xT_pool,
            tc.tile_pool(name="phA", bufs=4) as phA,
            tc.tile_pool(name="wchunk", bufs=2) as wchunk,
            tc.tile_pool(name="wtmp", bufs=2) as wtmp,
            tc.tile_pool(name="qk", bufs=8) as qkp,
            tc.tile_pool(name="sacc", bufs=1) as sacc,
            tc.tile_pool(name="soft", bufs=2) as soft,
            tc.tile_pool(name="vpool", bufs=3) as vpool,
            tc.tile_pool(name="opool", bufs=3) as opool,
            tc.tile_pool(name="ps_big", bufs=3, space="PSUM") as ps_big,
            tc.tile_pool(name="ps_s", bufs=2, space="PSUM") as ps_s,
            tc.tile_pool(name="ps_t", bufs=2, space="PSUM") as ps_t,
            tc.tile_pool(name="ps_warm", bufs=1, space="PSUM") as ps_warm,
        ):
            # --- constants ---
            mask_sb = const.tile([P, P], F32)
            nc.sync.dma_start(mask_sb, maskt[:])
            ident_sb = const.tile([P, P], BF16)
            nc.sync.dma_start(ident_sb, ident[:])
            bq_sb = const.tile([P, KO], F32)
            nc.sync.dma_start(bq_sb, bqt[:])
            bk_sb = const.tile([P, KO], F32)
            nc.sync.dma_start(bk_sb, bkt[:])
            bv_sb = const.tile([P, D], F32)
            nc.sync.dma_start(bv_sb, bvr[:])

            # HAM warm-up: dependency-free matmuls keep the PE clock gate
            # at full rate through the DMA-bound startup window, so real
            # work starts warm instead of paying the 1.2 GHz ramp.
            for _ in range(60):
                wps = ps_warm.tile([P, P], F32, tag="warm", name="wps")
                nc.tensor.matmul(wps, lhsT=ident_sb, rhs=ident_sb, start=True, stop=True)

            # persistent intermediates
            # xT[bi][p, ko, r] = x[row0 + r, ko*128 + p]  (bf16)
            xT = [
                xT_pool.tile([P, KO, nrows], BF16, name=f"xT{bi}")
                for bi, (_, nrows) in enumerate(BLOCKS)
            ]
            # S accumulator: S_all[p, i, :] for global 128-row subtile i
            S_all = sacc.tile([P, R // P, P], F32, name="S_all")
            # transposed softmax weights (lhsT for the O matmul)
            wT_all = sacc.tile([P, R // P, P], BF16, name="wT_all")

            def load_w_chunk(w_ap, col0, width, tag):
                dst = wchunk.tile([P, KO, width], BF16, tag=tag)
                # group k-tiles per DMA so the innermost run is the full
                # chunk width (1-2KB segments instead of 512B)
                kg = max(1, 2048 // width)  # 8KB fp32 staging per DMA
                for k0 in range(0, KO, kg):
                    tmp = wtmp.tile([P, kg, width], F32, tag="wtmp", name="wtmp")
                    nc.sync.dma_start(
                        tmp, w_ap[:, k0 : k0 + kg, col0 : col0 + width]
                    )
                    nc.vector.tensor_copy(dst[:, k0 : k0 + kg, :], tmp)
                return dst

            # Hoist the first W chunk loads so their DMAs start immediately.
            wq_tiles = {0: load_w_chunk(wq_ap, *CHUNKS1[0], "wq")}
            wk_tiles = {0: load_w_chunk(wk_ap, *CHUNKS1[0], "wk")}
            wv_tiles = {}

            # --- phase A: x -> bf16, PE-transpose into xT ---
            # Emission order sets DMA-ring order: interleave the pass-1 W
            # prefetches between phase-A blocks so neither starves the other.
            def phase_a_block(bi):
                row0, nrows = BLOCKS[bi]
                for s in range(4):  # 512-col strips of d_in, low k first
                    for rt in range(nrows // P):  # 128-row strips
                        r0 = row0 + rt * P
                        xt = phA.tile([P, 512], F32, tag="xt")
                        nc.sync.dma_start(
                            xt, xs[r0 : r0 + P, s * 512 : (s + 1) * 512]
                        )
                        xb = phA.tile([P, 512], BF16, tag="xb")
                        nc.vector.tensor_copy(xb, xt)
                        for t in range(4):  # 128-col tiles -> transpose
                            kt = s * 4 + t
                            pst = ps_t.tile([P, P], BF16, tag="tr")
                            nc.tensor.transpose(pst, xb[:, t * P : (t + 1) * P], ident_sb)
                            nc.vector.tensor_copy(
                                xT[bi][:, kt, rt * P : (rt + 1) * P], pst
                            )

            phase_a_block(0)
            wq_tiles[1] = load_w_chunk(wq_ap, *CHUNKS1[1], "wq")
            wk_tiles[1] = load_w_chunk(wk_ap, *CHUNKS1[1], "wk")
            phase_a_block(1)
            phase_a_block(2)
            wv_tiles[0] = load_w_chunk(wv_ap, *CHUNKS2[0], "wv")
            phase_a_block(3)

            # --- pass 1: qT/kT GEMMs + S accumulation ---
            pending_s = None  # (first, bi, qts, kts) awaiting S matmuls

            def emit_s(first, bi, qts, kts):
                row0, nrows = BLOCKS[bi]
                for sub in range(nrows // P):
                    pss = ps_s.tile([P, P], F32, tag="pss")
                    for jj in range(len(qts)):
                        nc.tensor.matmul(
                            pss,
                            lhsT=qts[jj][:, sub * P : (sub + 1) * P],
                            rhs=kts[jj][:, sub * P : (sub + 1) * P],
                            start=(jj == 0),
                            stop=(jj == len(qts) - 1),
                        )
                    i = row0 // P + sub
                    if first:
                        nc.vector.tensor_copy(S_all[:, i, :], pss)
                    else:
                        nc.vector.tensor_add(S_all[:, i, :], S_all[:, i, :], pss)

            for c, (col0, width) in enumerate(CHUNKS1):
                # one-chunk emission lookahead keeps the next chunk's DMAs
                # ahead of this chunk's compute in the rings
                if c + 1 < len(CHUNKS1) and (c + 1) not in wq_tiles:
                    wq_tiles[c + 1] = load_w_chunk(wq_ap, *CHUNKS1[c + 1], "wq")
                    wk_tiles[c + 1] = load_w_chunk(wk_ap, *CHUNKS1[c + 1], "wk")
                wq_sb = wq_tiles.pop(c)
                wk_sb = wk_tiles.pop(c)
                for bi, (row0, nrows) in enumerate(BLOCKS):
                    qts, kts = [], []
                    for jj in range(width // P):
                        j = col0 // P + jj
                        psq = ps_big.tile([P, 512], F32, tag="ps_big", name="psq")[:, :nrows]
                        for kt in range(KO):
                            nc.tensor.matmul(
                                psq,
                                lhsT=wq_sb[:, kt, jj * P : (jj + 1) * P],
                                rhs=xT[bi][:, kt, :],
                                start=(kt == 0),
                                stop=(kt == KO - 1),
                            )
                        qt = qkp.tile([P, 512], BF16, tag="qk", name="qt")[:, :nrows]
                        nc.scalar.activation(
                            qt, psq, mybir.ActivationFunctionType.Identity,
                            bias=bq_sb[:, j : j + 1],
                        )
                        qts.append(qt)
                        psk = ps_big.tile([P, 512], F32, tag="ps_big", name="psk")[:, :nrows]
                        for kt in range(KO):
                            nc.tensor.matmul(
                                psk,
                                lhsT=wk_sb[:, kt, jj * P : (jj + 1) * P],
                                rhs=xT[bi][:, kt, :],
                                start=(kt == 0),
                                stop=(kt == KO - 1),
                            )
                        ktile = qkp.tile([P, 512], BF16, tag="qk", name="ktile")[:, :nrows]
                        nc.scalar.activation(
                            ktile, psk, mybir.ActivationFunctionType.Identity,
                            bias=bk_sb[:, j : j + 1],
                        )
                        kts.append(ktile)
                    if pending_s is not None:
                        emit_s(*pending_s)
                    pending_s = (c == 0, bi, qts, kts)
            if pending_s is not None:
                emit_s(*pending_s)
                pending_s = None

            # --- softmax + transpose of one weight tile ---
            def emit_softmax(i):
                tmask = soft.tile([P, P], F32, tag="tmask")
                nc.vector.tensor_add(tmask, S_all[:, i, :], mask_sb)
                e = soft.tile([P, P], F32, tag="e")
                ssum = soft.tile([P, 1], F32, tag="ssum")
                nc.scalar.activation(
                    e, tmask, mybir.ActivationFunctionType.Exp,
                    scale=float(SCALE), accum_out=ssum,
                )
                rcp = soft.tile([P, 1], F32, tag="rcp")
                nc.vector.reciprocal(rcp, ssum)
                wsb = soft.tile([P, P], BF16, tag="wsb")
                nc.vector.tensor_scalar_mul(wsb, e, rcp)
                pst = ps_t.tile([P, P], BF16, tag="tr")
                nc.tensor.transpose(pst, wsb, ident_sb)
                nc.vector.tensor_copy(wT_all[:, i, :], pst)

            # --- pass 2: V GEMM + O = w @ V + bv ---
            # softmax for tile i is interleaved after the c=0 V chain for i,
            # so the PE streams V matmuls while DVE/ACT run the softmax.
            pending_o = None  # (v_sb, i, col0, width)

            def emit_o(v_sb, i, col0, width):
                pso = ps_big.tile([P, 512], F32, tag="ps_big", name="pso")[:, :width]
                nc.tensor.matmul(
                    pso, lhsT=wT_all[:, i, :], rhs=v_sb, start=True, stop=True
                )
                o_sb = opool.tile([P, 512], F32, tag="o", name="o_sb")[:, :width]
                nc.vector.tensor_add(o_sb, pso, bv_sb[:, col0 : col0 + width])
                r0 = i * P
                nc.sync.dma_start(out[r0 : r0 + P, col0 : col0 + width], o_sb)

            for c, (col0, width) in enumerate(CHUNKS2):
                if c + 1 < len(CHUNKS2) and (c + 1) not in wv_tiles:
                    wv_tiles[c + 1] = load_w_chunk(wv_ap, *CHUNKS2[c + 1], "wv")
                wv_sb = wv_tiles.pop(c)
                for bi, (row0, nrows) in enumerate(BLOCKS):
                    for rs in range(nrows // P):
                        i = row0 // P + rs
                        psv = ps_big.tile([P, 512], F32, tag="ps_big", name="psv")[:, :width]
                        for kt in range(KO):
                            nc.tensor.matmul(
                                psv,
                                lhsT=xT[bi][:, kt, rs * P : (rs + 1) * P],
                                rhs=wv_sb[:, kt, :],
                                start=(kt == 0),
                                stop=(kt == KO - 1),
                            )
                        v_sb = vpool.tile([P, 512], BF16, tag="v", name="v_sb")[:, :width]
                        nc.vector.tensor_copy(v_sb, psv)
                        if c == 0:
                            emit_softmax(i)
                        if pending_o is not None:
                            emit_o(*pending_o)
                        pending_o = (v_sb, i, col0, width)
            if pending_o is not None:
                emit_o(*pending_o)
                pending_o = None

    nc.compile()
    return nc


_CACHED = {}


def host_constants():
    mask = np.full((P, P), -1e9, dtype=np.float32)
    for g in range(P // H):
        mask[g * H : (g + 1) * H, g * H : (g + 1) * H] = 0.0
    identity = np.eye(P, dtype=ml_dtypes.bfloat16)
    return mask, identity


def kernel(x, Wq, bq, Wk, bk, Wv, bv):
    x = np.ascontiguousarray(np.asarray(x, dtype=np.float32))
    Wq = np.ascontiguousarray(np.asarray(Wq, dtype=np.float32))
    Wk = np.ascontiguousarray(np.asarray(Wk, dtype=np.float32))
    Wv = np.ascontiguousarray(np.asarray(Wv, dtype=np.float32))
    bq = np.asarray(bq, dtype=np.float32)
    bk = np.asarray(bk, dtype=np.float32)
    bv = np.asarray(bv, dtype=np.float32)

    if "nc" not in _CACHED:
        _CACHED["nc"] = build_program()
    nc = _CACHED["nc"]

    mask, identity = host_constants()
    bqt = np.ascontiguousarray(bq.reshape(KO, P).T)
    bkt = np.ascontiguousarray(bk.reshape(KO, P).T)
    bvr = np.ascontiguousarray(np.broadcast_to(bv, (P, D)))

    in_maps = []
    for i in range(N_CORES):
        in_maps.append(
            {
                "xs": x[i * R : (i + 1) * R],
                "Wq": Wq, "Wk": Wk, "Wv": Wv,
                "bqt": bqt, "bkt": bkt, "bvr": bvr,
                "maskt": mask, "ident": identity,
            }
        )
    res = run_bass_kernel_spmd(nc, in_maps, list(range(N_CORES)))
    return np.concatenate([res.results[i]["out"] for i in range(N_CORES)], axis=0)



# revision 3
# speedup vs baseline: 1.1189x; 1.1189x over previous
"""Trainium2 Bass kernel for nn_MultiHeadAttention_63986422775834.

Computation (see harness reference):
    q = x @ Wq + bq; k = x @ Wk + bk; v = x @ Wv + bv          # [N, D]
    group rows by 8: scores[b,h,g] = q[8b+h] . k[8b+g] / sqrt(D)
    w = softmax(scores, axis=-1);  out[8b+h] = sum_g w[b,h,g] * v[8b+g]

Key algebraic reduction: softmax is over the group axis g, so terms of
q.k^T that are constant along g cancel.  With M = Wq Wk^T and
v2 = Wk bq:
    softmax(q k^T) == softmax(t' x^T)  where t' = x M + 1 v2^T
(the x Wq bk^T and bq bk^T terms are g-constant; bq.(x Wk) folds into
the per-partition bias v2 of the t' GEMM).  M is row-count independent,
so its 2048^3 cost is computed ONCE, sharded over the 8 cores (each
computes a 256-row strip from host-pretransposed WqT/WkT), and shared
via an HBM AllGather.  Per-core tensor work drops from 3 big GEMMs
(q,k,v projections) to 2 (t' and v) + 1/8th of M.

Sharding: data-parallel over rows across 8 NeuronCores (2048 rows each;
row groups of 8 never cross a shard boundary).  Weights replicated
except the WqT 256-column strip.  Host pre-transposes/casts x and the
weights to bf16 (so no on-device transposes of x or W are needed) and
upcasts the bf16 output back to fp32.

Per-core phase order (chosen so the AllGather and the output drain are
both hidden behind GEMM work):
  M phase   : M[256-strip, :] = WqTs^T @ WkT   (65K PE cycles)
  AllGather : 1 MB -> 8 MB bf16 in DRAM (runs under the V phase)
  V phase a : v = x Wv + bv for d_out 0:1024, kept resident in SBUF
  t phase   : t' = x M + v2 streamed in 512-col chunks; S = t' x^T
              accumulated per 128-row tile; softmax interleaved into
              the last chunk's pipeline
  tail      : O = w V for resident half, then V chunks 2,3 streamed
              with O + output DMA interleaved (drain hidden)
"""

import sys

sys.path.insert(0, "/opt/trn_rl_repo")

import numpy as np
import ml_dtypes

import concourse.mybir as mybir
import concourse.tile as tile
from concourse import bacc
from concourse.bass_utils import run_bass_kernel_spmd

# problem shape (hardcoded per contract)
N_FULL = 16384
D = 2048
H = 8
N_CORES = 8
R = N_FULL // N_CORES  # rows per core = 2048
P = 128
KO = D // P  # 16 k-subtiles along the contraction dim
MS = D // N_CORES  # 256-row M strip per core
SCALE = 1.0 / np.sqrt(np.float32(D))

BF16 = mybir.dt.bfloat16
F32 = mybir.dt.float32

BLOCKS = [(0, 512), (512, 512), (1024, 512), (1536, 512)]  # row blocks
CW = 512  # chunk width for all streamed weight/M chunks
NC_CHUNKS = D // CW  # 4
V_RES = 2  # V chunks computed early and kept resident (d_out 0:1024)


def build_program():
    nc = bacc.Bacc("TRN2", target_bir_lowering=False, debug=False, num_devices=N_CORES)

    xsT = nc.dram_tensor("xsT", [D, R], BF16, kind="ExternalInput")
    WqTs = nc.dram_tensor("WqTs", [D, MS], BF16, kind="ExternalInput")
    WkT = nc.dram_tensor("WkT", [D, D], BF16, kind="ExternalInput")
    Wv = nc.dram_tensor("Wv", [D, D], BF16, kind="ExternalInput")
    v2t = nc.dram_tensor("v2t", [P, KO], F32, kind="ExternalInput")
    bvr = nc.dram_tensor("bvr", [P, D], BF16, kind="ExternalInput")
    maskt = nc.dram_tensor("maskt", [P, P], F32, kind="ExternalInput")
    ident = nc.dram_tensor("ident", [P, P], BF16, kind="ExternalInput")
    out = nc.dram_tensor("out", [R, D], BF16, kind="ExternalOutput")

    msh = nc.dram_tensor("msh", [MS, D], BF16)  # this core's M strip
    gath = nc.dram_tensor("gath", [D, D], BF16, addr_space="Shared")  # full M

    # partition-sliced views: t[p, ko, n] = T[ko*128 + p, n]
    xsT_ap = xsT[:].rearrange("(ko p) n -> p ko n", p=P)
    wqts_ap = WqTs[:].rearrange("(ko p) n -> p ko n", p=P)
    wkT_ap = WkT[:].rearrange("(ko p) n -> p ko n", p=P)
    wv_ap = Wv[:].rearrange("(ko p) n -> p ko n", p=P)
    m_ap = gath[:].rearrange("(ko p) n -> p ko n", p=P)

    with tile.TileContext(nc) as tc:
        with (
            tc.tile_pool(name="const", bufs=1) as const,
            tc.tile_pool(name="xT", bufs=1) as xT_pool,
            tc.tile_pool(name="vres", bufs=1) as vres,
            tc.tile_pool(name="wqts", bufs=1) as wqtsp,
            tc.tile_pool(name="wchunk", bufs=3) as wchunk,
            tc.tile_pool(name="qk", bufs=8) as qkp,
            tc.tile_pool(name="sacc", bufs=1) as sacc,
            tc.tile_pool(name="soft", bufs=2) as soft,
            tc.tile_pool(name="vpool", bufs=3) as vpool,
            tc.tile_pool(name="obuf", bufs=3) as obuf,
            tc.tile_pool(name="ps_big", bufs=3, space="PSUM") as ps_big,
            tc.tile_pool(name="ps_s", bufs=2, space="PSUM") as ps_s,
            tc.tile_pool(name="ps_t", bufs=2, space="PSUM") as ps_t,
            tc.tile_pool(name="ps_warm", bufs=1, space="PSUM") as ps_warm,
        ):
            # --- constants ---
            mask_sb = const.tile([P, P], F32)
            nc.sync.dma_start(mask_sb, maskt[:])
            ident_sb = const.tile([P, P], BF16)
            nc.sync.dma_start(ident_sb, ident[:])
            v2_sb = const.tile([P, KO], F32)
            nc.sync.dma_start(v2_sb, v2t[:])
            bv_sb = const.tile([P, D], BF16)
            nc.sync.dma_start(bv_sb, bvr[:])

            # HAM warm-up: dependency-free matmuls keep the PE clock gate
            # at full rate through the DMA-bound startup window.
            for _ in range(60):
                wps = ps_warm.tile([P, P], F32, tag="warm", name="wps")
                nc.tensor.matmul(wps, lhsT=ident_sb, rhs=ident_sb, start=True, stop=True)

            def load_chunk(ap, col0, width):
                dst = wchunk.tile([P, KO, CW], BF16, tag="w", name="wchunk")[:, :, :width]
                nc.sync.dma_start(dst, ap[:, :, col0 : col0 + width])
                return dst

            # --- hoisted DMAs (emission order = ring priority) ---
            wqts_sb = wqtsp.tile([P, KO, MS], BF16)
            nc.sync.dma_start(wqts_sb, wqts_ap[:, :, :])
            wkt_tiles = {0: load_chunk(wkT_ap, 0, CW)}

            # resident x^T (bf16, d_in on partitions), loaded in row blocks
            xT = xT_pool.tile([P, KO, R], BF16, name="xT")
            nc.sync.dma_start(xT[:, :, 0:512], xsT_ap[:, :, 0:512])
            wkt_tiles[1] = load_chunk(wkT_ap, CW, CW)
            for r0 in (512, 1024, 1536):
                nc.sync.dma_start(xT[:, :, r0 : r0 + 512], xsT_ap[:, :, r0 : r0 + 512])

            wv_tiles = {0: load_chunk(wv_ap, 0, CW)}

            # resident first-half V: V_all[p, i, d] = v[i*128 + p, d], d < 1024
            V_all = vres.tile([P, KO, V_RES * CW], BF16, name="V_all")

            # --- M phase: M[strip, :] = WqTs^T @ WkT ---
            for bc in range(NC_CHUNKS):
                if bc + 2 < NC_CHUNKS and (bc + 2) not in wkt_tiles:
                    wkt_tiles[bc + 2] = load_chunk(wkT_ap, (bc + 2) * CW, CW)
                wkt_sb = wkt_tiles.pop(bc)
                for ah in range(MS // P):
                    psm = ps_big.tile([P, CW], F32, tag="ps_big", name="psm")
                    for os_ in range(KO):
                        nc.tensor.matmul(
                            psm,
                            lhsT=wqts_sb[:, os_, ah * P : (ah + 1) * P],
                            rhs=wkt_sb[:, os_, :],
                            start=(os_ == 0),
                            stop=(os_ == KO - 1),
                        )
                    msb = obuf.tile([P, CW], BF16, tag="msh", name="msb")
                    nc.vector.tensor_copy(msb, psm)
                    nc.sync.dma_start(
                        msh[ah * P : (ah + 1) * P, bc * CW : (bc + 1) * CW], msb
                    )

            # --- AllGather the M strips (runs under the V phase) ---
            nc.gpsimd.collective_compute(
                "AllGather",
                mybir.AluOpType.bypass,
                replica_groups=[list(range(N_CORES))],
                ins=[msh[:]],
                outs=[gath[:]],
            )

            # --- V phase a: resident half ---
            for c in range(V_RES):
                if c + 1 < NC_CHUNKS and (c + 1) not in wv_tiles:
                    wv_tiles[c + 1] = load_chunk(wv_ap, (c + 1) * CW, CW)
                wv_sb = wv_tiles.pop(c)
                for rs in range(R // P):
                    psv = ps_big.tile([P, CW], F32, tag="ps_big", name="psv")
                    for kt in range(KO):
                        nc.tensor.matmul(
                            psv,
                            lhsT=xT[:, kt, rs * P : (rs + 1) * P],
                            rhs=wv_sb[:, kt, :],
                            start=(kt == 0),
                            stop=(kt == KO - 1),
                        )
                    nc.vector.tensor_copy(V_all[:, rs, c * CW : (c + 1) * CW], psv)

            # --- t phase: t' = x M + v2; S = t' x^T per 128-row tile ---
            m_tiles = {0: load_chunk(m_ap, 0, CW)}
            pending_s = None  # (first, bi, c, tts)

            def emit_s(first, bi, c, tts):
                row0, nrows = BLOCKS[bi]
                for sub in range(nrows // P):
                    pss = ps_s.tile([P, P], F32, tag="pss")
                    for jj in range(len(tts)):
                        nc.tensor.matmul(
                            pss,
                            lhsT=tts[jj][:, sub * P : (sub + 1) * P],
                            rhs=xT[
                                :,
                                c * (CW // P) + jj,
                                row0 + sub * P : row0 + (sub + 1) * P,
                            ],
                            start=(jj == 0),
                            stop=(jj == len(tts) - 1),
                        )
                    i = row0 // P + sub
                    if first:
                        nc.vector.tensor_copy(S_all[:, i, :], pss)
                    else:
                        nc.vector.tensor_add(S_all[:, i, :], S_all[:, i, :], pss)

            # S accumulator + transposed softmax weights
            S_all = sacc.tile([P, R // P, P], F32, name="S_all")
            wT_all = sacc.tile([P, R // P, P], BF16, name="wT_all")

            def emit_softmax(i):
                tmask = soft.tile([P, P], F32, tag="tmask")
                nc.vector.tensor_add(tmask, S_all[:, i, :], mask_sb)
                e = soft.tile([P, P], F32, tag="e")
                ssum = soft.tile([P, 1], F32, tag="ssum")
                nc.scalar.activation(
                    e, tmask, mybir.ActivationFunctionType.Exp,
                    scale=float(SCALE), accum_out=ssum,
                )
                rcp = soft.tile([P, 1], F32, tag="rcp")
                nc.vector.reciprocal(rcp, ssum)
                wsb = soft.tile([P, P], BF16, tag="wsb")
                nc.vector.tensor_scalar_mul(wsb, e, rcp)
                pst = ps_t.tile([P, P], BF16, tag="tr")
                nc.tensor.transpose(pst, wsb, ident_sb)
                nc.vector.tensor_copy(wT_all[:, i, :], pst)

            for c in range(NC_CHUNKS):
                if c + 1 < NC_CHUNKS and (c + 1) not in m_tiles:
                    m_tiles[c + 1] = load_chunk(m_ap, (c + 1) * CW, CW)
                m_sb = m_tiles.pop(c)
                for bi, (row0, nrows) in enumerate(BLOCKS):
                    tts = []
                    for jj in range(CW // P):
                        j = c * (CW // P) + jj
                        pst_ = ps_big.tile([P, CW], F32, tag="ps_big", name="psq")
                        for kt in range(KO):
                            nc.tensor.matmul(
                                pst_,
                                lhsT=m_sb[:, kt, jj * P : (jj + 1) * P],
                                rhs=xT[:, kt, row0 : row0 + nrows],
                                start=(kt == 0),
                                stop=(kt == KO - 1),
                            )
                        tt = qkp.tile([P, CW], BF16, tag="qk", name="tt")
                        nc.scalar.activation(
                            tt, pst_, mybir.ActivationFunctionType.Identity,
                            bias=v2_sb[:, j : j + 1],
                        )
                        tts.append(tt)
                    if pending_s is not None:
                        emit_s(*pending_s)
                        if pending_s[2] == NC_CHUNKS - 1:  # S final for that block
                            for sub in range(4):
                                emit_softmax(pending_s[1] * 4 + sub)
                    pending_s = (c == 0, bi, c, tts)
            if pending_s is not None:
                emit_s(*pending_s)
                for sub in range(4):
                    emit_softmax(pending_s[1] * 4 + sub)
                pending_s = None

            # --- tail: O = w V + bv ---
            def emit_o(v_src, i, col0, width):
                pso = ps_big.tile([P, CW], F32, tag="ps_big", name="pso")[:, :width]
                nc.tensor.matmul(
                    pso, lhsT=wT_all[:, i, :], rhs=v_src, start=True, stop=True
                )
                o_sb = obuf.tile([P, CW], BF16, tag="o", name="o_sb")[:, :width]
                nc.vector.tensor_add(o_sb, pso, bv_sb[:, col0 : col0 + width])
                r0 = i * P
                nc.sync.dma_start(out[r0 : r0 + P, col0 : col0 + width], o_sb)

            # resident half first (output drain starts immediately)
            for i in range(R // P):
                for c in range(V_RES):
                    emit_o(V_all[:, i, c * CW : (c + 1) * CW], i, c * CW, CW)

            # streamed second half, O interleaved per row tile
            pending_o = None
            for c in range(V_RES, NC_CHUNKS):
                if c + 1 < NC_CHUNKS and (c + 1) not in wv_tiles:
                    wv_tiles[c + 1] = load_chunk(wv_ap, (c + 1) * CW, CW)
                wv_sb = wv_tiles.pop(c)
                for rs in range(R // P):
                    psv = ps_big.tile([P, CW], F32, tag="ps_big", name="psv")
                    for kt in range(KO):
                        nc.tensor.matmul(
                            psv,
                            lhsT=xT[:, kt, rs * P : (rs + 1) * P],
                            rhs=wv_sb[:, kt, :],
                            start=(kt == 0),
                            stop=(kt == KO - 1),
                        )
                    v_sb = vpool.tile([P, CW], BF16, tag="v", name="v_sb")
                    nc.vector.tensor_copy(v_sb, psv)
                    if pending_o is not None:
                        emit_o(*pending_o)
                    pending_o = (v_sb, rs, c * CW, CW)
            if pending_o is not None:
                emit_o(*pending_o)
                pending_o = None

    nc.compile()
    return nc


_CACHED = {}


def host_constants():
    mask = np.full((P, P), -1e9, dtype=np.float32)
    for g in range(P // H):
        mask[g * H : (g + 1) * H, g * H : (g + 1) * H] = 0.0
    identity = np.eye(P, dtype=ml_dtypes.bfloat16)
    return mask, identity


def prepare_in_maps(x, Wq, bq, Wk, bk, Wv, bv):
    x = np.asarray(x, dtype=np.float32)
    Wq = np.asarray(Wq, dtype=np.float32)
    Wk = np.asarray(Wk, dtype=np.float32)
    Wv = np.asarray(Wv, dtype=np.float32)
    bq = np.asarray(bq, dtype=np.float32)
    bv = np.asarray(bv, dtype=np.float32)

    mask, identity = host_constants()
    xT_bf = np.ascontiguousarray(x.T.astype(ml_dtypes.bfloat16))  # [D, N]
    WqT_bf = np.ascontiguousarray(Wq.T.astype(ml_dtypes.bfloat16))
    WkT_bf = np.ascontiguousarray(Wk.T.astype(ml_dtypes.bfloat16))
    Wv_bf = np.ascontiguousarray(Wv.astype(ml_dtypes.bfloat16))
    v2 = (Wk @ bq).astype(np.float32)
    v2t = np.ascontiguousarray(v2.reshape(KO, P).T)
    bvr = np.ascontiguousarray(
        np.broadcast_to(bv.astype(ml_dtypes.bfloat16), (P, D))
    )

    in_maps = []
    for i in range(N_CORES):
        in_maps.append(
            {
                "xsT": np.ascontiguousarray(xT_bf[:, i * R : (i + 1) * R]),
                "WqTs": np.ascontiguousarray(WqT_bf[:, i * MS : (i + 1) * MS]),
                "WkT": WkT_bf,
                "Wv": Wv_bf,
                "v2t": v2t,
                "bvr": bvr,
                "maskt": mask,
                "ident": identity,
            }
        )
    return in_maps


def assemble_output(res):
    return np.concatenate(
        [res.results[i]["out"].astype(np.float32) for i in range(N_CORES)], axis=0
    )


def kernel(x, Wq, bq, Wk, bk, Wv, bv):
    if "nc" not in _CACHED:
        _CACHED["nc"] = build_program()
    nc = _CACHED["nc"]
    in_maps = prepare_in_maps(x, Wq, bq, Wk, bk, Wv, bv)
    res = run_bass_kernel_spmd(nc, in_maps, list(range(N_CORES)))
    return assemble_output(res)


# revision 7
# speedup vs baseline: 1.1377x; 1.0168x over previous
"""Trainium2 Bass kernel for nn_MultiHeadAttention_63986422775834.

Computation (see harness reference):
    q = x @ Wq + bq; k = x @ Wk + bk; v = x @ Wv + bv          # [N, D]
    group rows by 8: scores[b,h,g] = q[8b+h] . k[8b+g] / sqrt(D)
    w = softmax(scores, axis=-1);  out[8b+h] = sum_g w[b,h,g] * v[8b+g]

Key algebraic reduction: softmax is over the group axis g, so terms of
q.k^T that are constant along g cancel.  With M = Wq Wk^T and
v2 = Wk bq:
    softmax(q k^T) == softmax(t' x^T)  where t' = x M + 1 v2^T
(x Wq bk^T and bq bk^T are g-constant; bq.(x Wk) folds into the
per-partition bias v2 of the t' GEMM).  M is row-count independent, so
its 2048^3 cost is computed ONCE, sharded over the 8 cores (each core
computes a 256-row strip from host-pretransposed WqT/WkT) and shared
via an HBM AllGather.  Per-core tensor work drops from 3 big GEMMs to
2 + 1/8th of M.

Sharding: data-parallel over rows across 8 NeuronCores (2048 rows each;
row groups of 8 never cross a shard boundary).  Host pre-transposes and
casts x / weights to bf16 (no on-device transposes needed) and upcasts
the bf16 output back to fp32.

Measured HW notes driving the structure:
 - One engine queue issues DMAs serially at ~230-250 GB/s; emission
   order IS the schedule.  WkT chunks go first (the M phase consumes
   them at exactly the DMA rate), then xT / Wv; dependency-free filler
   matmuls bridge the DMA-bound gap between the M and V phases.
 - Issuing a collective drops the PE duty-cycle cap from 15/16 to
   13/16 for the remainder of the kernel (HAM type-31), so the AG is
   issued as early as possible and everything it gates is minimized.
 - S accumulates directly in PSUM (mask preloaded, all matmuls
   accumulate); softmax runs per-block inside the t phase.
 - The tail streams V chunks 2,3 and interleaves the resident-half O
   matmuls between chains as PE filler while output DMA drains.

Per-core phase order:
  M phase   : M[256-strip, :] = WqTs^T @ WkT   (65K PE cycles)
  AllGather : 1 MB -> 8 MB bf16 in DRAM (hidden under the V phase)
  V phase   : v = x Wv + bv for d_out 0:1024, kept resident in SBUF
  t phase   : t' = x M + v2 streamed in 512-col chunks; S += t' x^T
              into PSUM per 128-row tile; per-block softmax
  tail      : O = w V; resident-half O interleaved with streamed V
              chunks 2,3; bf16 output DMA overlapped
"""

import sys

sys.path.insert(0, "/opt/trn_rl_repo")

import numpy as np
import ml_dtypes

import concourse.mybir as mybir
import concourse.tile as tile
from concourse import bacc
from concourse.bass_utils import run_bass_kernel_spmd

# problem shape (hardcoded per contract)
N_FULL = 16384
D = 2048
H = 8
N_CORES = 8
R = N_FULL // N_CORES  # rows per core = 2048
P = 128
KO = D // P  # 16 k-subtiles along the contraction dim
MS = D // N_CORES  # 256-row M strip per core
SCALE = 1.0 / np.sqrt(np.float32(D))

BF16 = mybir.dt.bfloat16
F32 = mybir.dt.float32

BLOCKS = [(0, 512), (512, 512), (1024, 512), (1536, 512)]  # row blocks
CW = 512  # chunk width for all streamed weight/M chunks
NC_CHUNKS = D // CW  # 4
V_RES = 2  # V chunks computed early and kept resident (d_out 0:1024)
NT = R // P  # 16 row tiles per core


def build_program():
    nc = bacc.Bacc("TRN2", target_bir_lowering=False, debug=False, num_devices=N_CORES)

    xsT = nc.dram_tensor("xsT", [D, R], BF16, kind="ExternalInput")
    WqTs = nc.dram_tensor("WqTs", [D, MS], BF16, kind="ExternalInput")
    WkT = nc.dram_tensor("WkT", [D, D], BF16, kind="ExternalInput")
    Wv = nc.dram_tensor("Wv", [D, D], BF16, kind="ExternalInput")
    v2t = nc.dram_tensor("v2t", [P, KO], F32, kind="ExternalInput")
    bvr = nc.dram_tensor("bvr", [P, D], BF16, kind="ExternalInput")
    maskt = nc.dram_tensor("maskt", [P, P], F32, kind="ExternalInput")
    ident = nc.dram_tensor("ident", [P, P], BF16, kind="ExternalInput")
    out = nc.dram_tensor("out", [R, D], BF16, kind="ExternalOutput")

    msh = nc.dram_tensor("msh", [MS, D], BF16)  # this core's M strip
    gath = nc.dram_tensor("gath", [D, D], BF16, addr_space="Shared")  # full M

    # partition-sliced views: t[p, ko, n] = T[ko*128 + p, n]
    xsT_ap = xsT[:].rearrange("(ko p) n -> p ko n", p=P)
    wqts_ap = WqTs[:].rearrange("(ko p) n -> p ko n", p=P)
    wkT_ap = WkT[:].rearrange("(ko p) n -> p ko n", p=P)
    wv_ap = Wv[:].rearrange("(ko p) n -> p ko n", p=P)
    m_ap = gath[:].rearrange("(ko p) n -> p ko n", p=P)

    with tile.TileContext(nc) as tc:
        with (
            tc.tile_pool(name="const", bufs=1) as const,
            tc.tile_pool(name="xT", bufs=1) as xT_pool,
            tc.tile_pool(name="vres", bufs=1) as vres,
            tc.tile_pool(name="wqts", bufs=1) as wqtsp,
            tc.tile_pool(name="wchunk", bufs=3) as wchunk,
            tc.tile_pool(name="qk", bufs=8) as qkp,
            tc.tile_pool(name="sacc", bufs=1) as sacc,
            tc.tile_pool(name="soft", bufs=2) as soft,
            tc.tile_pool(name="vpool", bufs=3) as vpool,
            tc.tile_pool(name="obuf", bufs=3) as obuf,
            tc.tile_pool(name="ps_big", bufs=3, space="PSUM") as ps_big,
            tc.tile_pool(name="ps_sacc", bufs=1, space="PSUM") as ps_sacc,
            tc.tile_pool(name="ps_t", bufs=1, space="PSUM") as ps_t,
        ):
            # --- constants ---
            mask_sb = const.tile([P, P], F32)
            nc.sync.dma_start(mask_sb, maskt[:])
            ident_sb = const.tile([P, P], BF16)
            nc.sync.dma_start(ident_sb, ident[:])
            v2_sb = const.tile([P, KO], F32)
            nc.sync.dma_start(v2_sb, v2t[:])
            bv_sb = const.tile([P, D], BF16)
            nc.sync.dma_start(bv_sb, bvr[:])

            # HAM warm-up: dependency-free matmuls keep the PE busy/full-rate
            # through the DMA-bound startup window.
            for _ in range(60):
                wps = ps_big.tile([P, CW], F32, tag="ps_big", name="wps")[:, :P]
                nc.tensor.matmul(wps, lhsT=ident_sb, rhs=ident_sb, start=True, stop=True)

            def load_chunk(ap, col0, width):
                dst = wchunk.tile([P, KO, CW], BF16, tag="w", name="wchunk")[:, :, :width]
                nc.sync.dma_start(dst, ap[:, :, col0 : col0 + width])
                return dst

            # --- hoisted DMAs (one serial queue; emission order = schedule).
            # The M phase consumes WkT at the DMA rate, so all its chunks go
            # first; xT/Wv follow and are covered by M compute + filler.
            wqts_sb = wqtsp.tile([P, KO, MS], BF16)
            nc.sync.dma_start(wqts_sb, wqts_ap[:, :, :])
            wkt_tiles = {c: load_chunk(wkT_ap, c * CW, CW) for c in range(NC_CHUNKS)}

            xT = xT_pool.tile([P, KO, R], BF16, name="xT")
            nc.sync.dma_start(xT[:, :, 0:512], xsT_ap[:, :, 0:512])
            wv_tiles = {0: load_chunk(wv_ap, 0, CW)}
            for r0 in (512, 1024, 1536):
                nc.sync.dma_start(xT[:, :, r0 : r0 + 512], xsT_ap[:, :, r0 : r0 + 512])

            # resident first-half V: V_all[p, i, d] = v[i*128 + p, d], d < 1024
            V_all = vres.tile([P, KO, V_RES * CW], BF16, name="V_all")

            # --- M phase: M[strip, :] = WqTs^T @ WkT ---
            for bc in range(NC_CHUNKS):
                wkt_sb = wkt_tiles.pop(bc)
                for ah in range(MS // P):
                    psm = ps_big.tile([P, CW], F32, tag="ps_big", name="psm")
                    for os_ in range(KO):
                        nc.tensor.matmul(
                            psm,
                            lhsT=wqts_sb[:, os_, ah * P : (ah + 1) * P],
                            rhs=wkt_sb[:, os_, :],
                            start=(os_ == 0),
                            stop=(os_ == KO - 1),
                        )
                    msb = obuf.tile([P, CW], BF16, tag="msh", name="msb")
                    nc.vector.tensor_copy(msb, psm)
                    nc.sync.dma_start(
                        msh[ah * P : (ah + 1) * P, bc * CW : (bc + 1) * CW], msb
                    )

            # --- AllGather the M strips (completes under the V phase) ---
            nc.gpsimd.collective_compute(
                "AllGather",
                mybir.AluOpType.bypass,
                replica_groups=[list(range(N_CORES))],
                ins=[msh[:]],
                outs=[gath[:]],
            )

            # filler: bridges the DMA-bound gap until xT/wv0 arrive
            for _ in range(64):
                wps = ps_big.tile([P, CW], F32, tag="ps_big", name="wps")[:, :P]
                nc.tensor.matmul(wps, lhsT=ident_sb, rhs=ident_sb, start=True, stop=True)

            # --- V phase: resident half ---
            for c in range(V_RES):
                if c + 1 < NC_CHUNKS and (c + 1) not in wv_tiles:
                    wv_tiles[c + 1] = load_chunk(wv_ap, (c + 1) * CW, CW)
                wv_sb = wv_tiles.pop(c)
                for rs in range(NT):
                    psv = ps_big.tile([P, CW], F32, tag="ps_big", name="psv")
                    for kt in range(KO):
                        nc.tensor.matmul(
                            psv,
                            lhsT=xT[:, kt, rs * P : (rs + 1) * P],
                            rhs=wv_sb[:, kt, :],
                            start=(kt == 0),
                            stop=(kt == KO - 1),
                        )
                    nc.vector.tensor_copy(V_all[:, rs, c * CW : (c + 1) * CW], psv)

            # --- t phase: t' = x M + v2; S += t' x^T in PSUM ---
            # S_ps[p, i, :] is row-tile i's score tile; mask preloaded so every
            # S matmul accumulates (start=False).
            S_ps = ps_sacc.tile([P, NT, P], F32, name="S_ps")
            for i in range(NT):
                nc.vector.tensor_copy(S_ps[:, i, :], mask_sb)

            wT_all = sacc.tile([P, NT, P], BF16, name="wT_all")

            def emit_s(bi, c, tts):
                row0, nrows = BLOCKS[bi]
                for sub in range(nrows // P):
                    i = row0 // P + sub
                    for jj in range(len(tts)):
                        nc.tensor.matmul(
                            S_ps[:, i, :],
                            lhsT=tts[jj][:, sub * P : (sub + 1) * P],
                            rhs=xT[
                                :,
                                c * (CW // P) + jj,
                                row0 + sub * P : row0 + (sub + 1) * P,
                            ],
                            start=False,
                            stop=(c == NC_CHUNKS - 1 and jj == len(tts) - 1),
                        )

            def emit_softmax(i):
                e = soft.tile([P, P], F32, tag="e")
                ssum = soft.tile([P, 1], F32, tag="ssum")
                nc.scalar.activation(
                    e, S_ps[:, i, :], mybir.ActivationFunctionType.Exp,
                    scale=float(SCALE), accum_out=ssum,
                )
                rcp = soft.tile([P, 1], F32, tag="rcp")
                nc.vector.reciprocal(rcp, ssum)
                wsb = soft.tile([P, P], BF16, tag="wsb")
                nc.vector.tensor_scalar_mul(wsb, e, rcp)
                pst = ps_t.tile([P, P], BF16, tag="tr")
                nc.tensor.transpose(pst, wsb, ident_sb)
                nc.vector.tensor_copy(wT_all[:, i, :], pst)

            m_tiles = {0: load_chunk(m_ap, 0, CW)}
            pending_s = None  # (bi, c, tts)
            for c in range(NC_CHUNKS):
                if c + 1 < NC_CHUNKS and (c + 1) not in m_tiles:
                    m_tiles[c + 1] = load_chunk(m_ap, (c + 1) * CW, CW)
                m_sb = m_tiles.pop(c)
                for bi, (row0, nrows) in enumerate(BLOCKS):
                    tts = []
                    for jj in range(CW // P):
                        j = c * (CW // P) + jj
                        psq = ps_big.tile([P, CW], F32, tag="ps_big", name="psq")
                        for kt in range(KO):
                            nc.tensor.matmul(
                                psq,
                                lhsT=m_sb[:, kt, jj * P : (jj + 1) * P],
                                rhs=xT[:, kt, row0 : row0 + nrows],
                                start=(kt == 0),
                                stop=(kt == KO - 1),
                            )
                        tt = qkp.tile([P, CW], BF16, tag="qk", name="tt")
                        nc.scalar.activation(
                            tt, psq, mybir.ActivationFunctionType.Identity,
                            bias=v2_sb[:, j : j + 1],
                        )
                        tts.append(tt)
                    if pending_s is not None:
                        emit_s(*pending_s)
                        if pending_s[1] == NC_CHUNKS - 1:  # S final for that block
                            for sub in range(4):
                                emit_softmax(pending_s[0] * 4 + sub)
                    pending_s = (bi, c, tts)
            if pending_s is not None:
                emit_s(*pending_s)
                for sub in range(4):
                    emit_softmax(pending_s[0] * 4 + sub)
                pending_s = None

            # --- tail: O = w V + bv ---
            def emit_o(v_src, i, col0, width):
                pso = ps_big.tile([P, CW], F32, tag="ps_big", name="pso")[:, :width]
                nc.tensor.matmul(
                    pso, lhsT=wT_all[:, i, :], rhs=v_src, start=True, stop=True
                )
                o_sb = obuf.tile([P, CW], BF16, tag="o", name="o_sb")[:, :width]
                nc.vector.tensor_add(o_sb, pso, bv_sb[:, col0 : col0 + width])
                r0 = i * P
                nc.sync.dma_start(out[r0 : r0 + P, col0 : col0 + width], o_sb)

            # streamed second half; the 32 resident-half O matmuls are
            # interleaved between chains as PE filler
            resident_os = [(i, cc) for i in range(NT) for cc in range(V_RES)]
            pending_o = None
            for c in range(V_RES, NC_CHUNKS):
                if c + 1 < NC_CHUNKS and (c + 1) not in wv_tiles:
                    wv_tiles[c + 1] = load_chunk(wv_ap, (c + 1) * CW, CW)
                wv_sb = wv_tiles.pop(c)
                for rs in range(NT):
                    psv = ps_big.tile([P, CW], F32, tag="ps_big", name="psv")
                    for kt in range(KO):
                        nc.tensor.matmul(
                            psv,
                            lhsT=xT[:, kt, rs * P : (rs + 1) * P],
                            rhs=wv_sb[:, kt, :],
                            start=(kt == 0),
                            stop=(kt == KO - 1),
                        )
                    v_sb = vpool.tile([P, CW], BF16, tag="v", name="v_sb")
                    nc.vector.tensor_copy(v_sb, psv)
                    for _ in range(2):
                        if resident_os:
                            i, cc = resident_os.pop(0)
                            emit_o(V_all[:, i, cc * CW : (cc + 1) * CW], i, cc * CW, CW)
                    if pending_o is not None:
                        emit_o(*pending_o)
                    pending_o = (v_sb, rs, c * CW, CW)
            while resident_os:
                i, cc = resident_os.pop(0)
                emit_o(V_all[:, i, cc * CW : (cc + 1) * CW], i, cc * CW, CW)
            if pending_o is not None:
                emit_o(*pending_o)
                pending_o = None

    nc.compile()
    return nc


_CACHED = {}


def host_constants():
    mask = np.full((P, P), -1e9, dtype=np.float32)
    for g in range(P // H):
        mask[g * H : (g + 1) * H, g * H : (g + 1) * H] = 0.0
    identity = np.eye(P, dtype=ml_dtypes.bfloat16)
    return mask, identity


def prepare_in_maps(x, Wq, bq, Wk, bk, Wv, bv):
    x = np.asarray(x, dtype=np.float32)
    Wq = np.asarray(Wq, dtype=np.float32)
    Wk = np.asarray(Wk, dtype=np.float32)
    Wv = np.asarray(Wv, dtype=np.float32)
    bq = np.asarray(bq, dtype=np.float32)
    bv = np.asarray(bv, dtype=np.float32)

    mask, identity = host_constants()
    xT_bf = np.ascontiguousarray(x.T.astype(ml_dtypes.bfloat16))  # [D, N]
    WqT_bf = np.ascontiguousarray(Wq.T.astype(ml_dtypes.bfloat16))
    WkT_bf = np.ascontiguousarray(Wk.T.astype(ml_dtypes.bfloat16))
    Wv_bf = np.ascontiguousarray(Wv.astype(ml_dtypes.bfloat16))
    v2 = (Wk @ bq).astype(np.float32)
    v2t = np.ascontiguousarray(v2.reshape(KO, P).T)
    bvr = np.ascontiguousarray(
        np.broadcast_to(bv.astype(ml_dtypes.bfloat16), (P, D))
    )

    in_maps = []
    for i in range(N_CORES):
        in_maps.append(
            {
                "xsT": np.ascontiguousarray(xT_bf[:, i * R : (i + 1) * R]),
                "WqTs": np.ascontiguousarray(WqT_bf[:, i * MS : (i + 1) * MS]),
                "WkT": WkT_bf,
                "Wv": Wv_bf,
                "v2t": v2t,
                "bvr": bvr,
                "maskt": mask,
                "ident": identity,
            }
        )
    return in_maps


def assemble_output(res):
    return np.concatenate(
        [res.results[i]["out"].astype(np.float32) for i in range(N_CORES)], axis=0
    )


def kernel(x, Wq, bq, Wk, bk, Wv, bv):
    if "nc" not in _CACHED:
        _CACHED["nc"] = build_program()
    nc = _CACHED["nc"]
    in_maps = prepare_in_maps(x, Wq, bq, Wk, bk, Wv, bv)
    res = run_bass_kernel_spmd(nc, in_maps, list(range(N_CORES)))
    return assemble_output(res)


# revision 9
# speedup vs baseline: 1.1674x; 1.0261x over previous
"""Trainium2 Bass kernel for nn_MultiHeadAttention_63986422775834.

Computation (see harness reference):
    q = x @ Wq + bq; k = x @ Wk + bk; v = x @ Wv + bv          # [N, D]
    group rows by 8: scores[b,h,g] = q[8b+h] . k[8b+g] / sqrt(D)
    w = softmax(scores, axis=-1);  out[8b+h] = sum_g w[b,h,g] * v[8b+g]

Key algebraic reduction: softmax is over the group axis g, so terms of
q.k^T that are constant along g cancel.  With M = Wq Wk^T and
v2 = Wk bq:
    softmax(q k^T) == softmax(t' x^T)  where t' = x M + 1 v2^T
(x Wq bk^T and bq bk^T are g-constant; bq.(x Wk) folds into the
per-partition bias v2 of the t' GEMM).  M is row-count independent, so
its 2048^3 cost is computed ONCE, sharded over the 8 cores (each core
computes a 256-row strip from host-pretransposed WqT/WkT) and shared
via an HBM AllGather.  Per-core tensor work drops from 3 big GEMMs to
2 + 1/8th of M.

Sharding: data-parallel over rows across 8 NeuronCores (2048 rows each;
row groups of 8 never cross a shard boundary).  Host pre-transposes and
casts x / weights to bf16 (no on-device transposes needed) and upcasts
the bf16 output back to fp32.

Measured HW notes driving the structure:
 - One engine queue issues DMAs serially at ~230-250 GB/s; emission
   order IS the schedule.  WkT chunks go first (the M phase consumes
   them at exactly the DMA rate), then xT / Wv; dependency-free filler
   matmuls bridge the DMA-bound gap between the M and V phases.
 - Issuing a collective drops the PE duty-cycle cap from 15/16 to
   13/16 for the remainder of the kernel (HAM type-31), so the AG is
   issued as early as possible and everything it gates is minimized.
 - S accumulates directly in PSUM (mask preloaded, all matmuls
   accumulate); softmax runs per-block inside the t phase.
 - The tail streams V chunks 2,3 and interleaves the resident-half O
   matmuls between chains as PE filler while output DMA drains.

Per-core phase order:
  M phase   : M[256-strip, :] = WqTs^T @ WkT   (65K PE cycles)
  AllGather : 1 MB -> 8 MB bf16 in DRAM (hidden under the V phase)
  V phase   : v = x Wv + bv for d_out 0:1024, kept resident in SBUF
  t phase   : t' = x M + v2 streamed in 512-col chunks; S += t' x^T
              into PSUM per 128-row tile; per-block softmax
  tail      : O = w V; resident-half O interleaved with streamed V
              chunks 2,3; bf16 output DMA overlapped
"""

import sys

sys.path.insert(0, "/opt/trn_rl_repo")

import numpy as np
import ml_dtypes

import concourse.mybir as mybir
import concourse.tile as tile
from concourse import bacc
from concourse.bass_utils import run_bass_kernel_spmd

# problem shape (hardcoded per contract)
N_FULL = 16384
D = 2048
H = 8
N_CORES = 8
R = N_FULL // N_CORES  # rows per core = 2048
P = 128
KO = D // P  # 16 k-subtiles along the contraction dim
MS = D // N_CORES  # 256-row M strip per core
SCALE = 1.0 / np.sqrt(np.float32(D))

BF16 = mybir.dt.bfloat16
F32 = mybir.dt.float32

BLOCKS = [(0, 512), (512, 512), (1024, 512), (1536, 512)]  # row blocks
CW = 512  # chunk width for all streamed weight/M chunks
NC_CHUNKS = D // CW  # 4
V_RES = 2  # V chunks computed early and kept resident (d_out 0:1024)
NT = R // P  # 16 row tiles per core


def build_program():
    nc = bacc.Bacc("TRN2", target_bir_lowering=False, debug=False, num_devices=N_CORES)

    xsT = nc.dram_tensor("xsT", [D, R], BF16, kind="ExternalInput")
    WqTs = nc.dram_tensor("WqTs", [D, MS], BF16, kind="ExternalInput")
    WkT = nc.dram_tensor("WkT", [D, D], BF16, kind="ExternalInput")
    Wv = nc.dram_tensor("Wv", [D, D], BF16, kind="ExternalInput")
    v2t = nc.dram_tensor("v2t", [P, KO], F32, kind="ExternalInput")
    bvr = nc.dram_tensor("bvr", [P, D], BF16, kind="ExternalInput")
    maskt = nc.dram_tensor("maskt", [P, P], F32, kind="ExternalInput")
    ident = nc.dram_tensor("ident", [P, P], BF16, kind="ExternalInput")
    out = nc.dram_tensor("out", [R, D], BF16, kind="ExternalOutput")

    msh = nc.dram_tensor("msh", [MS, D], BF16)  # this core's M strip
    gath = nc.dram_tensor("gath", [D, D], BF16, addr_space="Shared")  # full M

    # partition-sliced views: t[p, ko, n] = T[ko*128 + p, n]
    xsT_ap = xsT[:].rearrange("(ko p) n -> p ko n", p=P)
    wqts_ap = WqTs[:].rearrange("(ko p) n -> p ko n", p=P)
    wkT_ap = WkT[:].rearrange("(ko p) n -> p ko n", p=P)
    wv_ap = Wv[:].rearrange("(ko p) n -> p ko n", p=P)
    m_ap = gath[:].rearrange("(ko p) n -> p ko n", p=P)

    with tile.TileContext(nc) as tc:
        with (
            tc.tile_pool(name="const", bufs=1) as const,
            tc.tile_pool(name="xT", bufs=1) as xT_pool,
            tc.tile_pool(name="vres", bufs=1) as vres,
            tc.tile_pool(name="wqts", bufs=1) as wqtsp,
            tc.tile_pool(name="wchunk", bufs=4) as wchunk,
            tc.tile_pool(name="qk", bufs=8) as qkp,
            tc.tile_pool(name="sacc", bufs=1) as sacc,
            tc.tile_pool(name="soft", bufs=2) as soft,
            tc.tile_pool(name="vpool", bufs=3) as vpool,
            tc.tile_pool(name="obuf", bufs=3) as obuf,
            tc.tile_pool(name="ps_big", bufs=3, space="PSUM") as ps_big,
            tc.tile_pool(name="ps_sacc", bufs=1, space="PSUM") as ps_sacc,
            tc.tile_pool(name="ps_t", bufs=1, space="PSUM") as ps_t,
        ):
            # --- constants ---
            mask_sb = const.tile([P, P], F32)
            nc.sync.dma_start(mask_sb, maskt[:])
            ident_sb = const.tile([P, P], BF16)
            nc.sync.dma_start(ident_sb, ident[:])
            v2_sb = const.tile([P, KO], F32)
            nc.sync.dma_start(v2_sb, v2t[:])
            bv_sb = const.tile([P, D], BF16)
            nc.sync.dma_start(bv_sb, bvr[:])

            # HAM warm-up: dependency-free matmuls keep the PE busy/full-rate
            # through the DMA-bound startup window.
            for _ in range(60):
                wps = ps_big.tile([P, CW], F32, tag="ps_big", name="wps")[:, :P]
                nc.tensor.matmul(wps, lhsT=ident_sb, rhs=ident_sb, start=True, stop=True)

            def load_chunk(ap, col0, width):
                dst = wchunk.tile([P, KO, CW], BF16, tag="w", name="wchunk")[:, :, :width]
                nc.sync.dma_start(dst, ap[:, :, col0 : col0 + width])
                return dst

            # --- hoisted DMAs (one serial in-order queue; emission order =
            # schedule).  Interleaved so M-phase (wkt) and V-phase (xb/wv)
            # inputs arrive just in time and the PE alternates between the
            # two phases without idling.
            wqts_sb = wqtsp.tile([P, KO, MS], BF16)
            nc.sync.dma_start(wqts_sb, wqts_ap[:, :, :])
            wkt_tiles = {c: load_chunk(wkT_ap, c * CW, CW) for c in range(2)}

            # x^T row-block tiles: xT[bi][p, kt, r] = x[bi*512 + r, kt*128+p]
            xT = []

            def load_x_block(bi):
                t = xT_pool.tile([P, KO, 512], BF16, tag=f"xT{bi}", name="xTb")
                nc.sync.dma_start(t, xsT_ap[:, :, bi * 512 : (bi + 1) * 512])
                xT.append(t)

            load_x_block(0)
            wv_tiles = {0: load_chunk(wv_ap, 0, CW)}
            wkt_tiles[2] = load_chunk(wkT_ap, 2 * CW, CW)
            load_x_block(1)
            wkt_tiles[3] = load_chunk(wkT_ap, 3 * CW, CW)
            load_x_block(2)
            load_x_block(3)

            # resident first-half V: V_all[p, i, d] = v[i*128 + p, d], d < 1024
            V_all = vres.tile([P, KO, V_RES * CW], BF16, name="V_all")

            def xt_slice(rs):  # [P, KO, 128] view of row tile rs
                return xT[rs // 4][:, :, (rs % 4) * P : (rs % 4 + 1) * P]

            def emit_m_chunk(bc):
                wkt_sb = wkt_tiles.pop(bc)
                for ah in range(MS // P):
                    psm = ps_big.tile([P, CW], F32, tag="ps_big", name="psm")
                    for os_ in range(KO):
                        nc.tensor.matmul(
                            psm,
                            lhsT=wqts_sb[:, os_, ah * P : (ah + 1) * P],
                            rhs=wkt_sb[:, os_, :],
                            start=(os_ == 0),
                            stop=(os_ == KO - 1),
                        )
                    msb = obuf.tile([P, CW], BF16, tag="msh", name="msb")
                    nc.vector.tensor_copy(msb, psm)
                    nc.sync.dma_start(
                        msh[ah * P : (ah + 1) * P, bc * CW : (bc + 1) * CW], msb
                    )

            def emit_v_chain(c, rs, wv_sb):
                psv = ps_big.tile([P, CW], F32, tag="ps_big", name="psv")
                xs = xt_slice(rs)
                for kt in range(KO):
                    nc.tensor.matmul(
                        psv,
                        lhsT=xs[:, kt, :],
                        rhs=wv_sb[:, kt, :],
                        start=(kt == 0),
                        stop=(kt == KO - 1),
                    )
                return psv

            # --- interleaved M phase + V chunk 0 ---
            emit_m_chunk(0)
            emit_m_chunk(1)
            wv0_sb = wv_tiles.pop(0)
            for rs in range(4):
                psv = emit_v_chain(0, rs, wv0_sb)
                nc.vector.tensor_copy(V_all[:, rs, 0:CW], psv)
            emit_m_chunk(2)
            for rs in range(4, 8):
                psv = emit_v_chain(0, rs, wv0_sb)
                nc.vector.tensor_copy(V_all[:, rs, 0:CW], psv)
            emit_m_chunk(3)

            # --- AllGather the M strips (completes under the V phase) ---
            nc.gpsimd.collective_compute(
                "AllGather",
                mybir.AluOpType.bypass,
                replica_groups=[list(range(N_CORES))],
                ins=[msh[:]],
                outs=[gath[:]],
            )

            wv_tiles[1] = load_chunk(wv_ap, CW, CW)
            for rs in range(8, NT):
                psv = emit_v_chain(0, rs, wv0_sb)
                nc.vector.tensor_copy(V_all[:, rs, 0:CW], psv)
            wv1_sb = wv_tiles.pop(1)
            for rs in range(NT):
                psv = emit_v_chain(1, rs, wv1_sb)
                nc.vector.tensor_copy(V_all[:, rs, CW : 2 * CW], psv)

            # --- t phase: t' = x M + v2; S += t' x^T in PSUM ---
            # S_ps[p, i, :] is row-tile i's score tile; mask preloaded so every
            # S matmul accumulates (start=False).
            S_ps = ps_sacc.tile([P, NT, P], F32, name="S_ps")
            for i in range(NT):
                nc.vector.tensor_copy(S_ps[:, i, :], mask_sb)

            wT_all = sacc.tile([P, NT, P], BF16, name="wT_all")

            def emit_s(bi, c, tts):
                row0, nrows = BLOCKS[bi]
                for sub in range(nrows // P):
                    i = row0 // P + sub
                    for jj in range(len(tts)):
                        nc.tensor.matmul(
                            S_ps[:, i, :],
                            lhsT=tts[jj][:, sub * P : (sub + 1) * P],
                            rhs=xT[bi][
                                :, c * (CW // P) + jj, sub * P : (sub + 1) * P
                            ],
                            start=False,
                            stop=(c == NC_CHUNKS - 1 and jj == len(tts) - 1),
                        )

            def emit_softmax(i):
                e = soft.tile([P, P], F32, tag="e")
                ssum = soft.tile([P, 1], F32, tag="ssum")
                nc.scalar.activation(
                    e, S_ps[:, i, :], mybir.ActivationFunctionType.Exp,
                    scale=float(SCALE), accum_out=ssum,
                )
                rcp = soft.tile([P, 1], F32, tag="rcp")
                nc.vector.reciprocal(rcp, ssum)
                wsb = soft.tile([P, P], BF16, tag="wsb")
                nc.vector.tensor_scalar_mul(wsb, e, rcp)
                pst = ps_t.tile([P, P], BF16, tag="tr")
                nc.tensor.transpose(pst, wsb, ident_sb)
                nc.vector.tensor_copy(wT_all[:, i, :], pst)

            m_tiles = {0: load_chunk(m_ap, 0, CW)}
            pending_s = None  # (bi, c, tts)
            for c in range(NC_CHUNKS):
                if c + 1 < NC_CHUNKS and (c + 1) not in m_tiles:
                    m_tiles[c + 1] = load_chunk(m_ap, (c + 1) * CW, CW)
                if c == 1:
                    wv_tiles[2] = load_chunk(wv_ap, 2 * CW, CW)
                elif c == 2:
                    wv_tiles[3] = load_chunk(wv_ap, 3 * CW, CW)
                m_sb = m_tiles.pop(c)
                for bi, (row0, nrows) in enumerate(BLOCKS):
                    tts = []
                    for jj in range(CW // P):
                        j = c * (CW // P) + jj
                        psq = ps_big.tile([P, CW], F32, tag="ps_big", name="psq")
                        for kt in range(KO):
                            nc.tensor.matmul(
                                psq,
                                lhsT=m_sb[:, kt, jj * P : (jj + 1) * P],
                                rhs=xT[bi][:, kt, :],
                                start=(kt == 0),
                                stop=(kt == KO - 1),
                            )
                        tt = qkp.tile([P, CW], BF16, tag="qk", name="tt")
                        nc.scalar.activation(
                            tt, psq, mybir.ActivationFunctionType.Identity,
                            bias=v2_sb[:, j : j + 1],
                        )
                        tts.append(tt)
                    if pending_s is not None:
                        emit_s(*pending_s)
                        if pending_s[1] == NC_CHUNKS - 1:  # S final for that block
                            for sub in range(4):
                                emit_softmax(pending_s[0] * 4 + sub)
                    pending_s = (bi, c, tts)
            if pending_s is not None:
                emit_s(*pending_s)
                for sub in range(4):
                    emit_softmax(pending_s[0] * 4 + sub)
                pending_s = None

            # --- tail: O = w V + bv ---
            def emit_o(v_src, i, col0, width):
                pso = ps_big.tile([P, CW], F32, tag="ps_big", name="pso")[:, :width]
                nc.tensor.matmul(
                    pso, lhsT=wT_all[:, i, :], rhs=v_src, start=True, stop=True
                )
                o_sb = obuf.tile([P, CW], BF16, tag="o", name="o_sb")[:, :width]
                nc.vector.tensor_add(o_sb, pso, bv_sb[:, col0 : col0 + width])
                r0 = i * P
                nc.sync.dma_start(out[r0 : r0 + P, col0 : col0 + width], o_sb)

            # streamed second half; the 32 resident-half O matmuls are
            # interleaved between chains as PE filler
            resident_os = [(i, cc) for i in range(NT) for cc in range(V_RES)]
            pending_o = None
            for c in range(V_RES, NC_CHUNKS):
                wv_sb = wv_tiles.pop(c)
                for rs in range(NT):
                    psv = emit_v_chain(c, rs, wv_sb)
                    v_sb = vpool.tile([P, CW], BF16, tag="v", name="v_sb")
                    nc.vector.tensor_copy(v_sb, psv)
                    if resident_os:
                        i, cc = resident_os.pop(0)
                        emit_o(V_all[:, i, cc * CW : (cc + 1) * CW], i, cc * CW, CW)
                    if pending_o is not None:
                        emit_o(*pending_o)
                    pending_o = (v_sb, rs, c * CW, CW)
            while resident_os:
                i, cc = resident_os.pop(0)
                emit_o(V_all[:, i, cc * CW : (cc + 1) * CW], i, cc * CW, CW)
            if pending_o is not None:
                emit_o(*pending_o)
                pending_o = None

    nc.compile()
    return nc


_CACHED = {}


def host_constants():
    mask = np.full((P, P), -1e9, dtype=np.float32)
    for g in range(P // H):
        mask[g * H : (g + 1) * H, g * H : (g + 1) * H] = 0.0
    identity = np.eye(P, dtype=ml_dtypes.bfloat16)
    return mask, identity


def prepare_in_maps(x, Wq, bq, Wk, bk, Wv, bv):
    x = np.asarray(x, dtype=np.float32)
    Wq = np.asarray(Wq, dtype=np.float32)
    Wk = np.asarray(Wk, dtype=np.float32)
    Wv = np.asarray(Wv, dtype=np.float32)
    bq = np.asarray(bq, dtype=np.float32)
    bv = np.asarray(bv, dtype=np.float32)

    mask, identity = host_constants()
    xT_bf = np.ascontiguousarray(x.T.astype(ml_dtypes.bfloat16))  # [D, N]
    WqT_bf = np.ascontiguousarray(Wq.T.astype(ml_dtypes.bfloat16))
    WkT_bf = np.ascontiguousarray(Wk.T.astype(ml_dtypes.bfloat16))
    Wv_bf = np.ascontiguousarray(Wv.astype(ml_dtypes.bfloat16))
    v2 = (Wk @ bq).astype(np.float32)
    v2t = np.ascontiguousarray(v2.reshape(KO, P).T)
    bvr = np.ascontiguousarray(
        np.broadcast_to(bv.astype(ml_dtypes.bfloat16), (P, D))
    )

    in_maps = []
    for i in range(N_CORES):
        in_maps.append(
            {
                "xsT": np.ascontiguousarray(xT_bf[:, i * R : (i + 1) * R]),
                "WqTs": np.ascontiguousarray(WqT_bf[:, i * MS : (i + 1) * MS]),
                "WkT": WkT_bf,
                "Wv": Wv_bf,
                "v2t": v2t,
                "bvr": bvr,
                "maskt": mask,
                "ident": identity,
            }
        )
    return in_maps


def assemble_output(res):
    return np.concatenate(
        [res.results[i]["out"].astype(np.float32) for i in range(N_CORES)], axis=0
    )


def kernel(x, Wq, bq, Wk, bk, Wv, bv):
    if "nc" not in _CACHED:
        _CACHED["nc"] = build_program()
    nc = _CACHED["nc"]
    in_maps = prepare_in_maps(x, Wq, bq, Wk, bk, Wv, bv)
    res = run_bass_kernel_spmd(nc, in_maps, list(range(N_CORES)))
    return assemble_output(res)


# revision 11
# speedup vs baseline: 1.1896x; 1.0191x over previous
"""Trainium2 Bass kernel for nn_MultiHeadAttention_63986422775834.

Computation (see harness reference):
    q = x @ Wq + bq; k = x @ Wk + bk; v = x @ Wv + bv          # [N, D]
    group rows by 8: scores[b,h,g] = q[8b+h] . k[8b+g] / sqrt(D)
    w = softmax(scores, axis=-1);  out[8b+h] = sum_g w[b,h,g] * v[8b+g]

Key algebraic reduction: softmax is over the group axis g, so terms of
q.k^T that are constant along g cancel.  With M = Wq Wk^T and
v2 = Wk bq:
    softmax(q k^T) == softmax(t' x^T)  where t' = x M + 1 v2^T
(x Wq bk^T and bq bk^T are g-constant; bq.(x Wk) folds into the
per-partition bias v2 of the t' GEMM).  M is row-count independent, so
its 2048^3 cost is computed ONCE, sharded over the 8 cores (each core
computes a 256-row strip from host-pretransposed WqT/WkT) and shared
via an HBM AllGather.  Per-core tensor work drops from 3 big GEMMs to
2 + 1/8th of M.

Sharding: data-parallel over rows across 8 NeuronCores (2048 rows each;
row groups of 8 never cross a shard boundary).  Host pre-transposes and
casts x / weights to bf16 (no on-device transposes needed) and upcasts
the bf16 output back to fp32.

Measured HW notes driving the structure:
 - One engine queue issues DMAs serially at ~230-250 GB/s; emission
   order IS the schedule.  WkT chunks go first (the M phase consumes
   them at exactly the DMA rate), then xT / Wv; dependency-free filler
   matmuls bridge the DMA-bound gap between the M and V phases.
 - Issuing a collective drops the PE duty-cycle cap from 15/16 to
   13/16 for the remainder of the kernel (HAM type-31), so the AG is
   issued as early as possible and everything it gates is minimized.
 - S accumulates directly in PSUM (mask preloaded, all matmuls
   accumulate); softmax runs per-block inside the t phase.
 - The tail streams V chunks 2,3 and interleaves the resident-half O
   matmuls between chains as PE filler while output DMA drains.

Per-core phase order:
  M phase   : M[256-strip, :] = WqTs^T @ WkT   (65K PE cycles)
  AllGather : 1 MB -> 8 MB bf16 in DRAM (hidden under the V phase)
  V phase   : v = x Wv + bv for d_out 0:1024, kept resident in SBUF
  t phase   : t' = x M + v2 streamed in 512-col chunks; S += t' x^T
              into PSUM per 128-row tile; per-block softmax
  tail      : O = w V; resident-half O interleaved with streamed V
              chunks 2,3; bf16 output DMA overlapped
"""

import sys

sys.path.insert(0, "/opt/trn_rl_repo")

import numpy as np
import ml_dtypes

import concourse.mybir as mybir
import concourse.tile as tile
from concourse import bacc
from concourse.bass_utils import run_bass_kernel_spmd

# problem shape (hardcoded per contract)
N_FULL = 16384
D = 2048
H = 8
N_CORES = 8
R = N_FULL // N_CORES  # rows per core = 2048
P = 128
KO = D // P  # 16 k-subtiles along the contraction dim
MS = D // N_CORES  # 256-row M strip per core
SCALE = 1.0 / np.sqrt(np.float32(D))

BF16 = mybir.dt.bfloat16
F32 = mybir.dt.float32

BLOCKS = [(0, 512), (512, 512), (1024, 512), (1536, 512)]  # row blocks
CW = 512  # chunk width for all streamed weight/M chunks
NC_CHUNKS = D // CW  # 4
V_RES = 2  # V chunks computed early and kept resident (d_out 0:1024)
NT = R // P  # 16 row tiles per core


def build_program():
    nc = bacc.Bacc("TRN2", target_bir_lowering=False, debug=False, num_devices=N_CORES)

    xsT = nc.dram_tensor("xsT", [D, R], BF16, kind="ExternalInput")
    WqTs = nc.dram_tensor("WqTs", [D, MS], BF16, kind="ExternalInput")
    WkT = nc.dram_tensor("WkT", [D, D], BF16, kind="ExternalInput")
    Wv = nc.dram_tensor("Wv", [D, D], BF16, kind="ExternalInput")
    v2t = nc.dram_tensor("v2t", [P, KO], F32, kind="ExternalInput")
    bvr = nc.dram_tensor("bvr", [P, D], BF16, kind="ExternalInput")
    maskt = nc.dram_tensor("maskt", [P, P], F32, kind="ExternalInput")
    ident = nc.dram_tensor("ident", [P, P], BF16, kind="ExternalInput")
    out = nc.dram_tensor("out", [R, D], BF16, kind="ExternalOutput")

    msh = nc.dram_tensor("msh", [MS, D], BF16)  # this core's M strip
    gath = nc.dram_tensor("gath", [D, D], BF16, addr_space="Shared")  # full M

    # partition-sliced views: t[p, ko, n] = T[ko*128 + p, n]
    xsT_ap = xsT[:].rearrange("(ko p) n -> p ko n", p=P)
    wqts_ap = WqTs[:].rearrange("(ko p) n -> p ko n", p=P)
    wkT_ap = WkT[:].rearrange("(ko p) n -> p ko n", p=P)
    wv_ap = Wv[:].rearrange("(ko p) n -> p ko n", p=P)
    m_ap = gath[:].rearrange("(ko p) n -> p ko n", p=P)

    with tile.TileContext(nc) as tc:
        with (
            tc.tile_pool(name="const", bufs=1) as const,
            tc.tile_pool(name="xT", bufs=1) as xT_pool,
            tc.tile_pool(name="vres", bufs=1) as vres,
            tc.tile_pool(name="wqts", bufs=1) as wqtsp,
            tc.tile_pool(name="wchunk", bufs=4) as wchunk,
            tc.tile_pool(name="qk", bufs=8) as qkp,
            tc.tile_pool(name="sacc", bufs=1) as sacc,
            tc.tile_pool(name="soft", bufs=2) as soft,
            tc.tile_pool(name="vpool", bufs=3) as vpool,
            tc.tile_pool(name="obuf", bufs=3) as obuf,
            tc.tile_pool(name="ps_big", bufs=3, space="PSUM") as ps_big,
            tc.tile_pool(name="ps_sacc", bufs=1, space="PSUM") as ps_sacc,
            tc.tile_pool(name="ps_t", bufs=1, space="PSUM") as ps_t,
        ):
            # --- constants ---
            mask_sb = const.tile([P, P], F32)
            nc.sync.dma_start(mask_sb, maskt[:])
            ident_sb = const.tile([P, P], BF16)
            nc.sync.dma_start(ident_sb, ident[:])
            v2_sb = const.tile([P, KO], F32)
            nc.sync.dma_start(v2_sb, v2t[:])
            bv_sb = const.tile([P, D], BF16)
            nc.sync.dma_start(bv_sb, bvr[:])

            # HAM warm-up: dependency-free matmuls keep the PE busy/full-rate
            # through the DMA-bound startup window.
            for _ in range(100):
                wps = ps_big.tile([P, CW], F32, tag="ps_big", name="wps")[:, :P]
                nc.tensor.matmul(wps, lhsT=ident_sb, rhs=ident_sb, start=True, stop=True)

            def load_chunk(ap, col0, width):
                dst = wchunk.tile([P, KO, CW], BF16, tag="w", name="wchunk")[:, :, :width]
                nc.sync.dma_start(dst, ap[:, :, col0 : col0 + width])
                return dst

            # --- hoisted DMAs (one serial in-order queue; emission order =
            # schedule).  Interleaved so M-phase (wkt) and V-phase (xb/wv)
            # inputs arrive just in time and the PE alternates between the
            # two phases without idling.
            wqts_sb = wqtsp.tile([P, KO, MS], BF16)
            nc.sync.dma_start(wqts_sb, wqts_ap[:, :, :])
            wkt_tiles = {c: load_chunk(wkT_ap, c * CW, CW) for c in range(2)}

            # x^T row-block tiles: xT[bi][p, kt, r] = x[bi*512 + r, kt*128+p]
            xT = []

            def load_x_block(bi):
                t = xT_pool.tile([P, KO, 512], BF16, tag=f"xT{bi}", name="xTb")
                nc.sync.dma_start(t, xsT_ap[:, :, bi * 512 : (bi + 1) * 512])
                xT.append(t)

            load_x_block(0)
            wv_tiles = {0: load_chunk(wv_ap, 0, CW)}
            load_x_block(1)
            wkt_tiles[2] = load_chunk(wkT_ap, 2 * CW, CW)
            load_x_block(2)
            wkt_tiles[3] = load_chunk(wkT_ap, 3 * CW, CW)
            load_x_block(3)

            # resident first-half V: V_all[p, i, d] = v[i*128 + p, d], d < 1024
            V_all = vres.tile([P, KO, V_RES * CW], BF16, name="V_all")

            def xt_slice(rs):  # [P, KO, 128] view of row tile rs
                return xT[rs // 4][:, :, (rs % 4) * P : (rs % 4 + 1) * P]

            def emit_m_chunk(bc):
                wkt_sb = wkt_tiles.pop(bc)
                for ah in range(MS // P):
                    psm = ps_big.tile([P, CW], F32, tag="ps_big", name="psm")
                    for os_ in range(KO):
                        nc.tensor.matmul(
                            psm,
                            lhsT=wqts_sb[:, os_, ah * P : (ah + 1) * P],
                            rhs=wkt_sb[:, os_, :],
                            start=(os_ == 0),
                            stop=(os_ == KO - 1),
                        )
                    msb = obuf.tile([P, CW], BF16, tag="msh", name="msb")
                    nc.vector.tensor_copy(msb, psm)
                    nc.sync.dma_start(
                        msh[ah * P : (ah + 1) * P, bc * CW : (bc + 1) * CW], msb
                    )

            def emit_v_chain(c, rs, wv_sb):
                psv = ps_big.tile([P, CW], F32, tag="ps_big", name="psv")
                xs = xt_slice(rs)
                for kt in range(KO):
                    nc.tensor.matmul(
                        psv,
                        lhsT=xs[:, kt, :],
                        rhs=wv_sb[:, kt, :],
                        start=(kt == 0),
                        stop=(kt == KO - 1),
                    )
                return psv

            # --- interleaved M phase + V chunk 0 ---
            emit_m_chunk(0)
            emit_m_chunk(1)
            wv0_sb = wv_tiles.pop(0)
            for rs in range(4):
                psv = emit_v_chain(0, rs, wv0_sb)
                nc.vector.tensor_copy(V_all[:, rs, 0:CW], psv)
            emit_m_chunk(2)
            for rs in range(4, 8):
                psv = emit_v_chain(0, rs, wv0_sb)
                nc.vector.tensor_copy(V_all[:, rs, 0:CW], psv)
            emit_m_chunk(3)

            # --- AllGather the M strips (completes under the V phase) ---
            nc.gpsimd.collective_compute(
                "AllGather",
                mybir.AluOpType.bypass,
                replica_groups=[list(range(N_CORES))],
                ins=[msh[:]],
                outs=[gath[:]],
            )

            wv_tiles[1] = load_chunk(wv_ap, CW, CW)
            for rs in range(8, NT):
                psv = emit_v_chain(0, rs, wv0_sb)
                nc.vector.tensor_copy(V_all[:, rs, 0:CW], psv)
            wv1_sb = wv_tiles.pop(1)
            for rs in range(NT):
                psv = emit_v_chain(1, rs, wv1_sb)
                nc.vector.tensor_copy(V_all[:, rs, CW : 2 * CW], psv)

            # --- t phase: t' = x M + v2; S += t' x^T in PSUM ---
            # S_ps[p, i, :] is row-tile i's score tile; mask preloaded so every
            # S matmul accumulates (start=False).
            S_ps = ps_sacc.tile([P, NT, P], F32, name="S_ps")
            for i in range(NT):
                nc.vector.tensor_copy(S_ps[:, i, :], mask_sb)

            wT_all = sacc.tile([P, NT, P], BF16, name="wT_all")

            def emit_s(bi, c, tts):
                row0, nrows = BLOCKS[bi]
                for sub in range(nrows // P):
                    i = row0 // P + sub
                    for jj in range(len(tts)):
                        nc.tensor.matmul(
                            S_ps[:, i, :],
                            lhsT=tts[jj][:, sub * P : (sub + 1) * P],
                            rhs=xT[bi][
                                :, c * (CW // P) + jj, sub * P : (sub + 1) * P
                            ],
                            start=False,
                            stop=(c == NC_CHUNKS - 1 and jj == len(tts) - 1),
                        )

            def emit_softmax(i):
                e = soft.tile([P, P], F32, tag="e")
                ssum = soft.tile([P, 1], F32, tag="ssum")
                nc.scalar.activation(
                    e, S_ps[:, i, :], mybir.ActivationFunctionType.Exp,
                    scale=float(SCALE), accum_out=ssum,
                )
                rcp = soft.tile([P, 1], F32, tag="rcp")
                nc.vector.reciprocal(rcp, ssum)
                wsb = soft.tile([P, P], BF16, tag="wsb")
                nc.vector.tensor_scalar_mul(wsb, e, rcp)
                pst = ps_t.tile([P, P], BF16, tag="tr")
                nc.tensor.transpose(pst, wsb, ident_sb)
                nc.vector.tensor_copy(wT_all[:, i, :], pst)

            m_tiles = {0: load_chunk(m_ap, 0, CW)}
            pending_s = None  # (bi, c, tts)
            for c in range(NC_CHUNKS):
                if c + 1 < NC_CHUNKS and (c + 1) not in m_tiles:
                    m_tiles[c + 1] = load_chunk(m_ap, (c + 1) * CW, CW)
                if c == 1:
                    wv_tiles[2] = load_chunk(wv_ap, 2 * CW, CW)
                elif c == 2:
                    wv_tiles[3] = load_chunk(wv_ap, 3 * CW, CW)
                m_sb = m_tiles.pop(c)
                for bi, (row0, nrows) in enumerate(BLOCKS):
                    tts = []
                    for jj in range(CW // P):
                        j = c * (CW // P) + jj
                        psq = ps_big.tile([P, CW], F32, tag="ps_big", name="psq")
                        for kt in range(KO):
                            nc.tensor.matmul(
                                psq,
                                lhsT=m_sb[:, kt, jj * P : (jj + 1) * P],
                                rhs=xT[bi][:, kt, :],
                                start=(kt == 0),
                                stop=(kt == KO - 1),
                            )
                        tt = qkp.tile([P, CW], BF16, tag="qk", name="tt")
                        nc.scalar.activation(
                            tt, psq, mybir.ActivationFunctionType.Identity,
                            bias=v2_sb[:, j : j + 1],
                        )
                        tts.append(tt)
                    if pending_s is not None:
                        emit_s(*pending_s)
                        if pending_s[1] == NC_CHUNKS - 1:  # S final for that block
                            for sub in range(4):
                                emit_softmax(pending_s[0] * 4 + sub)
                    pending_s = (bi, c, tts)
            if pending_s is not None:
                emit_s(*pending_s)
                for sub in range(4):
                    emit_softmax(pending_s[0] * 4 + sub)
                pending_s = None

            # --- tail: O = w V + bv ---
            def emit_o(v_src, i, col0, width):
                pso = ps_big.tile([P, CW], F32, tag="ps_big", name="pso")[:, :width]
                nc.tensor.matmul(
                    pso, lhsT=wT_all[:, i, :], rhs=v_src, start=True, stop=True
                )
                o_sb = obuf.tile([P, CW], BF16, tag="o", name="o_sb")[:, :width]
                nc.vector.tensor_add(o_sb, pso, bv_sb[:, col0 : col0 + width])
                r0 = i * P
                nc.sync.dma_start(out[r0 : r0 + P, col0 : col0 + width], o_sb)

            # streamed second half; the 32 resident-half O matmuls are
            # interleaved between chains as PE filler
            resident_os = [(i, cc) for i in range(NT) for cc in range(V_RES)]
            pending_o = None
            for c in range(V_RES, NC_CHUNKS):
                wv_sb = wv_tiles.pop(c)
                for rs in range(NT):
                    psv = emit_v_chain(c, rs, wv_sb)
                    v_sb = vpool.tile([P, CW], BF16, tag="v", name="v_sb")
                    nc.vector.tensor_copy(v_sb, psv)
                    if resident_os:
                        i, cc = resident_os.pop(0)
                        emit_o(V_all[:, i, cc * CW : (cc + 1) * CW], i, cc * CW, CW)
                    if pending_o is not None:
                        emit_o(*pending_o)
                    pending_o = (v_sb, rs, c * CW, CW)
            while resident_os:
                i, cc = resident_os.pop(0)
                emit_o(V_all[:, i, cc * CW : (cc + 1) * CW], i, cc * CW, CW)
            if pending_o is not None:
                emit_o(*pending_o)
                pending_o = None

    nc.compile()
    return nc


_CACHED = {}


def host_constants():
    mask = np.full((P, P), -1e9, dtype=np.float32)
    for g in range(P // H):
        mask[g * H : (g + 1) * H, g * H : (g + 1) * H] = 0.0
    identity = np.eye(P, dtype=ml_dtypes.bfloat16)
    return mask, identity


def prepare_in_maps(x, Wq, bq, Wk, bk, Wv, bv):
    x = np.asarray(x, dtype=np.float32)
    Wq = np.asarray(Wq, dtype=np.float32)
    Wk = np.asarray(Wk, dtype=np.float32)
    Wv = np.asarray(Wv, dtype=np.float32)
    bq = np.asarray(bq, dtype=np.float32)
    bv = np.asarray(bv, dtype=np.float32)

    mask, identity = host_constants()
    xT_bf = np.ascontiguousarray(x.T.astype(ml_dtypes.bfloat16))  # [D, N]
    WqT_bf = np.ascontiguousarray(Wq.T.astype(ml_dtypes.bfloat16))
    WkT_bf = np.ascontiguousarray(Wk.T.astype(ml_dtypes.bfloat16))
    Wv_bf = np.ascontiguousarray(Wv.astype(ml_dtypes.bfloat16))
    v2 = (Wk @ bq).astype(np.float32)
    v2t = np.ascontiguousarray(v2.reshape(KO, P).T)
    bvr = np.ascontiguousarray(
        np.broadcast_to(bv.astype(ml_dtypes.bfloat16), (P, D))
    )

    in_maps = []
    for i in range(N_CORES):
        in_maps.append(
            {
                "xsT": np.ascontiguousarray(xT_bf[:, i * R : (i + 1) * R]),
                "WqTs": np.ascontiguousarray(WqT_bf[:, i * MS : (i + 1) * MS]),
                "WkT": WkT_bf,
                "Wv": Wv_bf,
                "v2t": v2t,
                "bvr": bvr,
                "maskt": mask,
                "ident": identity,
            }
        )
    return in_maps


def assemble_output(res):
    return np.concatenate(
        [res.results[i]["out"].astype(np.float32) for i in range(N_CORES)], axis=0
    )


def kernel(x, Wq, bq, Wk, bk, Wv, bv):
    if "nc" not in _CACHED:
        _CACHED["nc"] = build_program()
    nc = _CACHED["nc"]
    in_maps = prepare_in_maps(x, Wq, bq, Wk, bk, Wv, bv)
    res = run_bass_kernel_spmd(nc, in_maps, list(range(N_CORES)))
    return assemble_output(res)
